# revision 40
# baseline (speedup 1.0000x reference)
"""Masked-loss kernel for nn_MLoss_9715216024200 on 8 Trainium2 NeuronCores.

loss = sum(where(y[...,0]>0.5, (y-x)^2 - a*x^2, 0)) + a*sum(x[...,0]^2)
with x,y f32 (256, 10647, 5); output is a f32 scalar.

Sharding: flatten both tensors to cells (5 contiguous f32 each), pad with
256 zero-cells (mathematically neutral: y0=0 -> mask 0, x=0 -> no bg term),
reshape to (8 cores, 128 partitions, 2662 cells).  Each core streams its
13 MiB at the ~360 GB/s HBM roofline while three compute engines split the
elementwise work (every engine under the per-tile DMA time):

  per 127-cell tile (down to telescoped tail tiles):
    GpSimd: m5  = bf16(y0 > 0.5) replicated to all 5 features (contiguous)
            xs0 = bf16(sqrt(a)*x0)  -> tail slice of the group dmx buffer
            xm  = x * m5 for every 4th tile and the late tiles
    DVE:    d   = y - x   (f32 1x, bf16 out)
            dm  = d * m5  (bf16 2x) -> head slice of the group dmx buffer
            xm  = x * m5  (mixed 1x, bf16 out) for the remaining tiles
  per reduction GROUP of 1-3 consecutive tiles (ScalarE Square+accum_out,
  fp32 accumulate; grouping amortizes the ~370ns fixed cost per
  accumulate -- 187ns accumulator read + SBUF-access init):
            acc1[g] = sum(dmx^2) = sum((m*d)^2) + a*sum(x0^2)
            acc2[g] = sum(xm^2)  (unscaled; host applies a)
  the last two groups' acc2 run as DVE scalar_tensor_tensor accumulates,
  deferred past the loop, so the endgame after the final DMA is short.

m*v^2 == (m*v)^2 because m is 0/1, which is what lets the fused
Square-accumulate do all reductions.  Small tiles keep the DMA->accumulate
pipeline latency low (the last accumulate lands ~4.7us after the final
byte); grouped reductions keep ScalarE's fixed costs amortized.  bf16
intermediates cost ~2e-6 relative error on the final sum.  Host combines:
total = sum(acc1) - a*sum(acc2), in f64 over 8 cores x 128 partitions.
"""
import sys

for _p in ('/opt/trn_rl_repo',):
    if _p in sys.path:
        sys.path.remove(_p)
    sys.path.insert(0, _p)

import numpy as np

B, C, F = 256, 10647, 5
THRESH = 0.5
ALPHA = 0.1
N_CORES = 8
P = 128
CELLS = B * C                      # 2,725,632
CELLS_PER_PART = 2662              # ceil to 8*128*2662 = 2,725,888
PAD_CELLS = N_CORES * P * CELLS_PER_PART - CELLS   # 256
FD = CELLS_PER_PART * F            # 13310 elems per partition per core

TILE_SIZES = [123] + [127] * 15 + [218, 166, 125, 125]
assert sum(TILE_SIZES) == CELLS_PER_PART
N_TILES = len(TILE_SIZES)
# reduction groups over consecutive tiles (one sq/sq2 pair per group)
GROUP_OF = [3, 3, 3, 3, 2, 2, 1, 1, 1, 1]
assert sum(GROUP_OF) == N_TILES
N_GROUPS = len(GROUP_OF)
_tile_group = [(gi, k) for gi, gn in enumerate(GROUP_OF) for k in range(gn)]
XM_ON_POOL = {3, 7, 11, 15, 17, 19}   # tiles whose xm runs on GpSimd
M5_ON_DVE = {0}       # tile 0's mask on DVE (Pool's broadcast would gate
                      # the pipeline head)
SQ2_ON_DVE = set()    # mid groups: sq2 as DVE stt (unused in final config)
TTR_TAIL = 2          # last k groups: sq2 as deferred DVE stt
REV_TTR = False
BUFS = [8, 8, 8, 4]
import os as _os
# groups whose sq2 is split: first SPLIT_FRAC on ACT, rest as deferred DVE
# stt into an extra acc column
SPLIT_SQ2 = set(int(v) for v in _os.environ.get('SPLIT_SQ2', '').split(',') if v != '')
SPLIT_FRAC = float(_os.environ.get('SPLIT_FRAC', '0.5'))
N_EXTRA = len(SPLIT_SQ2)
# last k tiles: emit compute ops under tc.high_priority() so the Tile
# scheduler favors the endgame chain over deferred stts
HIPRI_TAIL = int(_os.environ.get('HIPRI_TAIL', '0'))

_compiled = None


def _build():
    from contextlib import ExitStack
    import concourse.tile as tile
    from concourse import bacc, mybir

    sqa = float(np.sqrt(ALPHA))

    nc = bacc.Bacc("TRN2", target_bir_lowering=False, debug=False,
                   enable_asserts=True, num_devices=N_CORES)
    x_d = nc.dram_tensor("x", [P, FD], mybir.dt.float32, kind="ExternalInput").ap()
    y_d = nc.dram_tensor("y", [P, FD], mybir.dt.float32, kind="ExternalInput").ap()
    o_d = nc.dram_tensor("o", [P, 2 * N_GROUPS + N_EXTRA], mybir.dt.float32,
                         kind="ExternalOutput").ap()

    f32 = mybir.dt.float32
    bf16 = mybir.dt.bfloat16
    Sq = mybir.ActivationFunctionType.Square
    Alu = mybir.AluOpType

    with tile.TileContext(nc) as tc, ExitStack() as ctx:
        xp = ctx.enter_context(tc.tile_pool(name="x", bufs=BUFS[0]))
        yp = ctx.enter_context(tc.tile_pool(name="y", bufs=BUFS[1]))
        wp = ctx.enter_context(tc.tile_pool(name="work", bufs=BUFS[2]))
        sp = ctx.enter_context(tc.tile_pool(name="scratch", bufs=BUFS[3]))
        tp = ctx.enter_context(tc.tile_pool(name="tailscratch", bufs=2))
        ap_ = ctx.enter_context(tc.tile_pool(name="acc", bufs=1))

        # interleaved acc layout: columns [2g, 2g+1] = (dm-side, xm-side);
        # extra trailing columns hold DVE halves of split sq2 reductions
        acc = ap_.tile([P, 2 * N_GROUPS + N_EXTRA], f32)

        tail_ttr = []
        tail_split = []
        off = 0
        gdmx = gxm = None
        gdoff = gxoff = 0
        from contextlib import nullcontext
        for t, cells in enumerate(TILE_SIZES):
            fd = cells * F
            g, k_in_g = _tile_group[t]
            gn = GROUP_OF[g]
            gcells = sum(TILE_SIZES[t - k_in_g:t - k_in_g + gn])
            hipri = (tc.high_priority() if t >= N_TILES - HIPRI_TAIL
                     else nullcontext())
            xt = xp.tile([P, fd], f32, tag="xt")
            yt = yp.tile([P, fd], f32, tag="yt")
            sl = slice(off, off + fd)
            off += fd
            nc.sync.dma_start(yt[:], y_d[:, sl])
            nc.sync.dma_start(xt[:], x_d[:, sl])

            if k_in_g == 0:
                # group buffers: dmx = [dm(t0)|xs0(t0)|dm(t1)|xs0(t1)|...],
                # gxm = [xm(t0)|xm(t1)|...]
                gdmx = wp.tile([P, (gcells * F) + gcells], bf16, tag="dmx")
                gxm = wp.tile([P, gcells * F], bf16, tag="xmg")
                gdoff = gxoff = 0

            # bf16 mask replicated to all 5 features (contiguous); emitted
            # before xs0 because dm (critical path) waits on it
            with hipri:
                m5 = wp.tile([P, fd], bf16, tag="m5")
                y0b = yt[:, 0::F].unsqueeze(2).broadcast_to((P, cells, F))
                m5_eng = nc.vector if t in M5_ON_DVE else nc.gpsimd
                m5_eng.tensor_scalar(
                    m5[:].rearrange("p (k f) -> p k f", f=F), y0b,
                    THRESH, None, op0=Alu.is_gt)

                # GpSimd: xs0 = sqrt(a)*x0 into this tile's dmx tail slice
                nc.gpsimd.tensor_scalar(
                    gdmx[:, gdoff + fd:gdoff + fd + cells], xt[:, 0::F],
                    sqa, None, op0=Alu.mult)

                # DVE: d = y-x (bf16 out), dm = d*m5 (bf16 2x), xm = x*m5
                dt_ = wp.tile([P, fd], bf16, tag="d")
                nc.vector.tensor_tensor(dt_[:], yt[:], xt[:],
                                        op=Alu.subtract)
                nc.vector.tensor_tensor(gdmx[:, gdoff:gdoff + fd], dt_[:],
                                        m5[:], op=Alu.mult)
                xm_eng = nc.gpsimd if t in XM_ON_POOL else nc.vector
                xm_eng.tensor_tensor(gxm[:, gxoff:gxoff + fd], xt[:], m5[:],
                                     op=Alu.mult)
            gdoff += fd + cells
            gxoff += fd

            if k_in_g == gn - 1:
                # group complete: fused square + row-sum over group buffers
                sq = sp.tile([P, gcells * F + gcells], bf16, tag="sq")
                nc.scalar.activation(sq[:], gdmx[:], Sq,
                                     accum_out=acc[:, 2 * g:2 * g + 1])
                if g in SPLIT_SQ2:
                    h = (int(gcells * F * SPLIT_FRAC) // F) * F
                    sq2 = sp.tile([P, h], bf16, tag="sq2")
                    nc.scalar.activation(sq2[:], gxm[:, 0:h], Sq,
                                         accum_out=acc[:, 2 * g + 1:2 * g + 2])
                    xcol = 2 * N_GROUPS + sorted(SPLIT_SQ2).index(g)
                    tail_split.append((g, gxm, h, gcells * F, xcol))
                elif g >= N_GROUPS - TTR_TAIL:
                    tail_ttr.append((g, gxm, gcells))
                elif g in SQ2_ON_DVE:
                    sq2 = sp.tile([P, gcells * F], bf16, tag="sq2")
                    nc.vector.scalar_tensor_tensor(
                        sq2[:], gxm[:], 1.0, gxm[:], op0=Alu.mult,
                        op1=Alu.mult, accum_out=acc[:, 2 * g + 1:2 * g + 2])
                else:
                    sq2 = sp.tile([P, gcells * F], bf16, tag="sq2")
                    nc.scalar.activation(sq2[:], gxm[:], Sq,
                                         accum_out=acc[:, 2 * g + 1:2 * g + 2])

        for (g, gxm, h, n, xcol) in tail_split:
            s2 = sp.tile([P, n - h], bf16, tag="sq2")
            nc.vector.scalar_tensor_tensor(
                s2[:], gxm[:, h:n], 1.0, gxm[:, h:n],
                op0=Alu.mult, op1=Alu.mult,
                accum_out=acc[:, xcol:xcol + 1])

        if REV_TTR:
            tail_ttr = tail_ttr[::-1]
        for (g, gxm, gc) in tail_ttr:
            # xm * 1 * xm summed per row == sum(xm^2); runs on DVE, deferred
            # past the loop so the last tiles' d/dm (which gate ACT) go first
            # (scalar_tensor_tensor is Pool-invalid but DVE-valid on HW).
            # Dedicated scratch pool: sharing sp would add a WAR wait on an
            # unrelated ACT op's scratch buffer to this DVE op
            sq2 = tp.tile([P, gc * F], bf16, tag="tsq2")
            nc.vector.scalar_tensor_tensor(
                sq2[:], gxm[:], 1.0, gxm[:],
                op0=Alu.mult, op1=Alu.mult, accum_out=acc[:, 2 * g + 1:2 * g + 2])

        nc.sync.dma_start(o_d[:], acc[:])

    nc.compile()
    return nc


def _shard(a: np.ndarray) -> list[np.ndarray]:
    flat = a.reshape(-1)
    pad = np.zeros(PAD_CELLS * F, dtype=a.dtype)
    flat = np.concatenate([flat, pad])
    per_core = flat.reshape(N_CORES, P, FD)
    return [np.ascontiguousarray(per_core[i]) for i in range(N_CORES)]


def kernel(x: np.ndarray, y: np.ndarray) -> np.ndarray:
    global _compiled
    if _compiled is None:
        _compiled = _build()
    nc = _compiled

    from concourse.bass_utils import run_bass_kernel_spmd

    xs = _shard(np.asarray(x, dtype=np.float32))
    ys = _shard(np.asarray(y, dtype=np.float32))
    in_maps = [{"x": xs[i], "y": ys[i]} for i in range(N_CORES)]
    res = run_bass_kernel_spmd(nc, in_maps, core_ids=list(range(N_CORES)))

    total = np.float64(0.0)
    for r in res.results:
        o = r["o"].astype(np.float64).reshape(P, 2 * N_GROUPS + N_EXTRA)
        total += o[:, 0:2 * N_GROUPS:2].sum()
        total -= ALPHA * o[:, 1:2 * N_GROUPS:2].sum()
        total -= ALPHA * o[:, 2 * N_GROUPS:].sum()
    return np.float32(total)


# revision 42
# speedup vs baseline: 1.0215x; 1.0215x over previous
"""Masked-loss kernel for nn_MLoss_9715216024200 on 8 Trainium2 NeuronCores.

loss = sum(where(y[...,0]>0.5, (y-x)^2 - a*x^2, 0)) + a*sum(x[...,0]^2)
with x,y f32 (256, 10647, 5); output is a f32 scalar.

Sharding: flatten both tensors to cells (5 contiguous f32 each), pad with
256 zero-cells (mathematically neutral: y0=0 -> mask 0, x=0 -> no bg term),
reshape to (8 cores, 128 partitions, 2662 cells).  Each core streams its
13 MiB at the ~360 GB/s HBM roofline while three compute engines split the
elementwise work (every engine under the per-tile DMA time):

  per 127-cell tile (down to telescoped tail tiles):
    GpSimd: m5  = bf16(y0 > 0.5) replicated to all 5 features (contiguous)
            xs0 = bf16(sqrt(a)*x0)  -> tail slice of the group dmx buffer
            xm  = x * m5 for every 4th tile and the late tiles
    DVE:    d   = y - x   (f32 1x, bf16 out)
            dm  = d * m5  (bf16 2x) -> head slice of the group dmx buffer
            xm  = x * m5  (mixed 1x, bf16 out) for the remaining tiles
  per reduction GROUP of 1-3 consecutive tiles (ScalarE Square+accum_out,
  fp32 accumulate; grouping amortizes the ~370ns fixed cost per
  accumulate -- 187ns accumulator read + SBUF-access init):
            acc1[g] = sum(dmx^2) = sum((m*d)^2) + a*sum(x0^2)
            acc2[g] = sum(xm^2)  (unscaled; host applies a)
  the last two groups' acc2 run as DVE scalar_tensor_tensor accumulates,
  deferred past the loop, so the endgame after the final DMA is short.

m*v^2 == (m*v)^2 because m is 0/1, which is what lets the fused
Square-accumulate do all reductions.  Small tiles keep the DMA->accumulate
pipeline latency low (the last accumulate lands ~4.7us after the final
byte); grouped reductions keep ScalarE's fixed costs amortized.  bf16
intermediates cost ~2e-6 relative error on the final sum.  Host combines:
total = sum(acc1) - a*sum(acc2), in f64 over 8 cores x 128 partitions.
"""
import sys

for _p in ('/opt/trn_rl_repo',):
    if _p in sys.path:
        sys.path.remove(_p)
    sys.path.insert(0, _p)

import numpy as np

B, C, F = 256, 10647, 5
THRESH = 0.5
ALPHA = 0.1
N_CORES = 8
P = 128
CELLS = B * C                      # 2,725,632
CELLS_PER_PART = 2662              # ceil to 8*128*2662 = 2,725,888
PAD_CELLS = N_CORES * P * CELLS_PER_PART - CELLS   # 256
FD = CELLS_PER_PART * F            # 13310 elems per partition per core

TILE_SIZES = [123] + [127] * 15 + [218, 166, 125, 125]
assert sum(TILE_SIZES) == CELLS_PER_PART
N_TILES = len(TILE_SIZES)
# reduction groups over consecutive tiles (one sq/sq2 pair per group)
GROUP_OF = [3, 3, 3, 3, 2, 2, 1, 1, 1, 1]
assert sum(GROUP_OF) == N_TILES
N_GROUPS = len(GROUP_OF)
_tile_group = [(gi, k) for gi, gn in enumerate(GROUP_OF) for k in range(gn)]
XM_ON_POOL = {3, 7, 11, 15, 17, 19}   # tiles whose xm runs on GpSimd
M5_ON_DVE = {0}       # tile 0's mask on DVE (Pool's broadcast would gate
                      # the pipeline head)
SQ2_ON_DVE = set()    # mid groups: sq2 as DVE stt (unused in final config)
TTR_TAIL = 2          # last k groups: sq2 as deferred DVE stt
REV_TTR = False
BUFS = [8, 8, 8, 4]

_compiled = None


def _build():
    from contextlib import ExitStack
    import concourse.tile as tile
    from concourse import bacc, mybir

    sqa = float(np.sqrt(ALPHA))

    nc = bacc.Bacc("TRN2", target_bir_lowering=False, debug=False,
                   enable_asserts=True, num_devices=N_CORES)
    x_d = nc.dram_tensor("x", [P, FD], mybir.dt.float32, kind="ExternalInput").ap()
    y_d = nc.dram_tensor("y", [P, FD], mybir.dt.float32, kind="ExternalInput").ap()
    o_d = nc.dram_tensor("o", [P, 2 * N_GROUPS], mybir.dt.float32,
                         kind="ExternalOutput").ap()

    f32 = mybir.dt.float32
    bf16 = mybir.dt.bfloat16
    Sq = mybir.ActivationFunctionType.Square
    Alu = mybir.AluOpType

    with tile.TileContext(nc) as tc, ExitStack() as ctx:
        xp = ctx.enter_context(tc.tile_pool(name="x", bufs=BUFS[0]))
        yp = ctx.enter_context(tc.tile_pool(name="y", bufs=BUFS[1]))
        wp = ctx.enter_context(tc.tile_pool(name="work", bufs=BUFS[2]))
        sp = ctx.enter_context(tc.tile_pool(name="scratch", bufs=BUFS[3]))
        ap_ = ctx.enter_context(tc.tile_pool(name="acc", bufs=1))

        # interleaved acc layout: columns [2g, 2g+1] = (dm-side, xm-side)
        acc = ap_.tile([P, 2 * N_GROUPS], f32)

        tail_ttr = []
        off = 0
        gdmx = gxm = None
        gdoff = gxoff = 0
        for t, cells in enumerate(TILE_SIZES):
            fd = cells * F
            g, k_in_g = _tile_group[t]
            gn = GROUP_OF[g]
            gcells = sum(TILE_SIZES[t - k_in_g:t - k_in_g + gn])
            xt = xp.tile([P, fd], f32, tag="xt")
            yt = yp.tile([P, fd], f32, tag="yt")
            sl = slice(off, off + fd)
            off += fd
            nc.sync.dma_start(yt[:], y_d[:, sl])
            nc.sync.dma_start(xt[:], x_d[:, sl])

            if k_in_g == 0:
                # group buffers: dmx = [dm(t0)|xs0(t0)|dm(t1)|xs0(t1)|...],
                # gxm = [xm(t0)|xm(t1)|...]
                gdmx = wp.tile([P, (gcells * F) + gcells], bf16, tag="dmx")
                gxm = wp.tile([P, gcells * F], bf16, tag="xmg")
                gdoff = gxoff = 0

            # bf16 mask replicated to all 5 features (contiguous); emitted
            # before xs0 because dm (critical path) waits on it
            m5 = wp.tile([P, fd], bf16, tag="m5")
            y0b = yt[:, 0::F].unsqueeze(2).broadcast_to((P, cells, F))
            m5_eng = nc.vector if t in M5_ON_DVE else nc.gpsimd
            m5_eng.tensor_scalar(
                m5[:].rearrange("p (k f) -> p k f", f=F), y0b,
                THRESH, None, op0=Alu.is_gt)

            # GpSimd: xs0 = sqrt(a)*x0 into this tile's dmx tail slice
            nc.gpsimd.tensor_scalar(
                gdmx[:, gdoff + fd:gdoff + fd + cells], xt[:, 0::F],
                sqa, None, op0=Alu.mult)

            # DVE: d = y - x (bf16 out), dm = d*m5 (bf16 2x), xm = x*m5
            dt_ = wp.tile([P, fd], bf16, tag="d")
            nc.vector.tensor_tensor(dt_[:], yt[:], xt[:], op=Alu.subtract)
            nc.vector.tensor_tensor(gdmx[:, gdoff:gdoff + fd], dt_[:], m5[:],
                                    op=Alu.mult)
            xm_eng = nc.gpsimd if t in XM_ON_POOL else nc.vector
            xm_eng.tensor_tensor(gxm[:, gxoff:gxoff + fd], xt[:], m5[:],
                                 op=Alu.mult)
            gdoff += fd + cells
            gxoff += fd

            if k_in_g == gn - 1:
                # group complete: fused square + row-sum over group buffers
                sq = sp.tile([P, gcells * F + gcells], bf16, tag="sq")
                nc.scalar.activation(sq[:], gdmx[:], Sq,
                                     accum_out=acc[:, 2 * g:2 * g + 1])
                if g >= N_GROUPS - TTR_TAIL:
                    tail_ttr.append((g, gxm, gcells))
                elif g in SQ2_ON_DVE:
                    sq2 = sp.tile([P, gcells * F], bf16, tag="sq2")
                    nc.vector.scalar_tensor_tensor(
                        sq2[:], gxm[:], 1.0, gxm[:], op0=Alu.mult,
                        op1=Alu.mult, accum_out=acc[:, 2 * g + 1:2 * g + 2])
                else:
                    sq2 = sp.tile([P, gcells * F], bf16, tag="sq2")
                    nc.scalar.activation(sq2[:], gxm[:], Sq,
                                         accum_out=acc[:, 2 * g + 1:2 * g + 2])

        if REV_TTR:
            tail_ttr = tail_ttr[::-1]
        for (g, gxm, gc) in tail_ttr:
            # xm * 1 * xm summed per row == sum(xm^2); runs on DVE, deferred
            # past the loop so the last tiles' d/dm (which gate ACT) go first
            # (scalar_tensor_tensor is Pool-invalid but DVE-valid on HW)
            sq2 = sp.tile([P, gc * F], bf16, tag="sq2")
            nc.vector.scalar_tensor_tensor(
                sq2[:], gxm[:], 1.0, gxm[:],
                op0=Alu.mult, op1=Alu.mult, accum_out=acc[:, 2 * g + 1:2 * g + 2])

        nc.sync.dma_start(o_d[:], acc[:])

    nc.compile()
    return nc


def _shard(a: np.ndarray) -> list[np.ndarray]:
    flat = a.reshape(-1)
    pad = np.zeros(PAD_CELLS * F, dtype=a.dtype)
    flat = np.concatenate([flat, pad])
    per_core = flat.reshape(N_CORES, P, FD)
    return [np.ascontiguousarray(per_core[i]) for i in range(N_CORES)]


def kernel(x: np.ndarray, y: np.ndarray) -> np.ndarray:
    global _compiled
    if _compiled is None:
        _compiled = _build()
    nc = _compiled

    from concourse.bass_utils import run_bass_kernel_spmd

    xs = _shard(np.asarray(x, dtype=np.float32))
    ys = _shard(np.asarray(y, dtype=np.float32))
    in_maps = [{"x": xs[i], "y": ys[i]} for i in range(N_CORES)]
    res = run_bass_kernel_spmd(nc, in_maps, core_ids=list(range(N_CORES)))

    total = np.float64(0.0)
    for r in res.results:
        o = r["o"].astype(np.float64).reshape(P, 2 * N_GROUPS)
        total += o[:, 0::2].sum()
        total -= ALPHA * o[:, 1::2].sum()
    return np.float32(total)
